# revision 18
# baseline (speedup 1.0000x reference)
"""Trainium2 Bass kernel for nn_M02SameVQ (VQ codebook match + refiner MLP).

v6-final (8 NeuronCores, data-parallel over batch; 849us -> 745us):
 - Coarse scores s = x.c in fp16 on the PE (fp32 PSUM accum), weight-
   stationary over two 4-bank PSUM phases (fewer weight reloads); the
   -0.5||c||^2 bias and the 1/8 score scale fuse into the DVE PSUM
   evacuation (scalar_tensor_tensor: out = psum*0.125 + bias_rep), so the
   PE runs no per-(slot,j) K=2 bias matmuls at all.
 - Top-8 scan (MAX8 + FIND_INDEX8) is emitted one slot behind the coarse
   matmuls so PSUM evacuations never queue behind a 4.4us scan on DVE.
 - Top-2 candidates rescued exactly: indirect row gathers from the fp32
   extended codebook, dots via GpSimd mul + DVE reduce (cand0) and a DVE
   fused mul+row-sum (cand1); winner selected in-place with a predicated
   copy, cast fp16 on ScalarE, moved channel-major by one xbar DMA
   transpose per slot on the Scalar HWDGE ring (feature loads and output
   stores ride the Sync ring, so a rescue-blocked transpose cannot
   head-of-line block them).
 - spk residual per slot: in-place fp16 subtract (2x DVE mode) + fp8 cast
   on ScalarE.
 - Refiner MLP in fp8(e4m3) DoubleRow perf mode (2x PE throughput);
   per-layer power-of-two activation scales and the x16 weight scale fold
   into the activation evacuations (leaky-relu is positively homogeneous);
   lin folds back in the output layer via a x256-scaled fp16 identity
   matmul and db2 is applied as a per-partition activation bias.
 - Accuracy: top-2-of-fp16 + exact fp32 re-score gives 0-1 wrong picks on
   this data (rel err 3.4e-4 measured, tolerance 2e-2); fp8 MLP error is
   negligible because ||spk_dec|| is only 0.3% of the output norm.
"""

import numpy as np
import concourse.tile as tile
import concourse.mybir as mybir
from concourse import bacc, bass, bass_utils
from concourse.masks import make_identity

F32 = mybir.dt.float32
F16 = mybir.dt.float16
F8 = mybir.dt.float8e4
U8 = mybir.dt.uint8
U32 = mybir.dt.uint32
AF = mybir.ActivationFunctionType
ALU = mybir.AluOpType
AX = mybir.AxisListType
PM = mybir.MatmulPerfMode
LRELU = AF.Lrelu

B, C, T = 16, 1024, 1500
NBINS, HID, EMB = 4096, 512, 256
NCORES = 8
BPC = B // NCORES          # batches per core
NT = 125                   # real tokens per slot
SLOT = 128                 # token slot width (3 pad columns)
NSLOT = 4                  # slots per block
BLKR = NT * NSLOT          # real tokens per block (500)
BLKW = SLOT * NSLOT        # slot columns per block (512)
NBLK = T // BLKR           # blocks per batch (3)
CE = 1088                  # ext codebook row: 1024 cen | -0.5||c||^2 | 1.0 | pad
KG = C // 128              # 8 contraction chunks
NJ = NBINS // 512          # 8 bin-tiles
NCAND = 2                  # rescued candidates (top-2; top-3 exact on this data)
WS = 16.0                  # fp8 weight scale
# per-layer fp8 input activation scales: spk, h1, h2, z, d1, d2
HS = [1.0, 4.0, 8.0, 16.0, 8.0, 16.0]

_CACHE = {}


def _build_body(nc, tc, d):
    with tc.tile_pool(name="const", bufs=1) as cp, \
         tc.tile_pool(name="work", bufs=1) as wp, \
         tc.tile_pool(name="psd", bufs=1, space="PSUM") as psd, \
         tc.tile_pool(name="psx", bufs=1, space="PSUM") as psx, \
         tc.tile_pool(name="psm", bufs=1, space="PSUM") as psm:

        # ---- resident constants (coarse-path first so PE can start early) ----
        b2rep = cp.tile([128, NBINS], F16)
        nc.scalar.dma_start(out=b2rep, in_=d["b2rep"])
        cs16 = []
        for q in range(4):
            cq = cp.tile([128, KG, 1024], F16, tag=f"cs{q}")
            eng = nc.scalar if q % 2 == 0 else nc.sync
            eng.dma_start(
                out=cq,
                in_=d["cT16"][:, q * 1024:(q + 1) * 1024]
                    .rearrange("(g p) b -> p g b", p=128))
            cs16.append(cq)
        ident = cp.tile([128, 128], F32)
        make_identity(nc, ident)
        ident16 = cp.tile([128, 128], F16)
        nc.vector.tensor_copy(out=ident16, in_=ident)
        idents = cp.tile([128, 128], F16)
        nc.scalar.activation(idents, ident, AF.Copy, scale=256.0)

        w0 = cp.tile([128, KG, HID], F8)
        nc.scalar.dma_start(out=w0, in_=d["w0T"].rearrange("(g p) h -> p g h", p=128))
        w1 = cp.tile([128, 4, HID], F8)
        nc.scalar.dma_start(out=w1, in_=d["w1T"].rearrange("(g p) h -> p g h", p=128))
        w2 = cp.tile([128, 4, EMB], F8)
        nc.scalar.dma_start(out=w2, in_=d["w2T"].rearrange("(g p) h -> p g h", p=128))
        v0 = cp.tile([128, 2, HID], F8)
        nc.scalar.dma_start(out=v0, in_=d["v0T"].rearrange("(g p) h -> p g h", p=128))
        v1 = cp.tile([128, 4, HID], F8)
        nc.scalar.dma_start(out=v1, in_=d["v1T"].rearrange("(g p) h -> p g h", p=128))
        v2 = cp.tile([128, 4, C], F8)
        nc.scalar.dma_start(out=v2, in_=d["v2T"].rearrange("(g p) h -> p g h", p=128))
        b0 = cp.tile([128, 4], F32)
        nc.scalar.dma_start(out=b0, in_=d["b0"].rearrange("(g p) -> p g", p=128))
        b1 = cp.tile([128, 4], F32)
        nc.scalar.dma_start(out=b1, in_=d["b1"].rearrange("(g p) -> p g", p=128))
        b2 = cp.tile([128, 2], F32)
        nc.scalar.dma_start(out=b2, in_=d["b2"].rearrange("(g p) -> p g", p=128))
        c0 = cp.tile([128, 4], F32)
        nc.scalar.dma_start(out=c0, in_=d["c0"].rearrange("(g p) -> p g", p=128))
        c1 = cp.tile([128, 4], F32)
        nc.scalar.dma_start(out=c1, in_=d["c1"].rearrange("(g p) -> p g", p=128))
        c2b = cp.tile([128, KG], F32)
        nc.scalar.dma_start(out=c2b, in_=d["c2b"].rearrange("(g p) -> p g", p=128))

        saved = {}
        fstate = {}

        def front_head(bi):
            batch, blk = divmod(bi, NBLK)
            feat_b = d["feat"][batch].rearrange("(g p) t -> p g t", p=128)
            fs16 = wp.tile([128, KG, BLKW], F16, tag="fs16", bufs=2)
            linT = wp.tile([128, KG, BLKW], F16, tag="linT", bufs=2)
            spk8 = wp.tile([128, KG, BLKW], F8, tag="spk8", bufs=2)
            state = {}
            fstate[bi] = (batch, blk, feat_b, fs16, linT, spk8, state)

            def stage_a1(t):
                tok0 = blk * BLKR + t * NT
                csl = slice(t * SLOT, t * SLOT + NT)
                fs32t = wp.tile([128, KG, NT], F32, tag="fs32t", bufs=2)
                nc.sync.dma_start(out=fs32t, in_=feat_b[:, :, tok0:tok0 + NT])
                nc.scalar.copy(fs16[:, :, csl], fs32t)

                # coarse fp16 scores, weight-stationary over 4-bank phases;
                # evac fuses 1/8 scale + bin bias on DVE
                s16 = wp.tile([NT, NBINS], F16, tag="s16", bufs=2)
                for ph in range(2):
                    pjs = []
                    for _i in range(4):
                        pj = psd.tile([NT, 512], F32, tag="dist", bufs=4)
                        pjs.append(pj)
                    for g in range(KG):
                        for i, pj in enumerate(pjs):
                            j = ph * 4 + i
                            nc.tensor.matmul(pj, lhsT=fs16[:, g, csl],
                                             rhs=cs16[j // 2][:, g,
                                                              (j % 2) * 512:
                                                              (j % 2) * 512 + 512],
                                             start=(g == 0), stop=(g == KG - 1))
                    for i, pj in enumerate(pjs):
                        j = ph * 4 + i
                        nc.vector.scalar_tensor_tensor(
                            out=s16[:, j * 512:(j + 1) * 512], in0=pj,
                            scalar=0.125, in1=b2rep[:NT, j * 512:(j + 1) * 512],
                            op0=ALU.mult, op1=ALU.add)

                # x_t via PE transpose, evacuated to SBUF
                xtp = psx.tile([NT, C], F32, tag="xt", bufs=1)
                for g in range(KG):
                    nc.tensor.transpose(xtp[:, g * 128:(g + 1) * 128],
                                        fs32t[:, g, :], ident)
                x32t = wp.tile([NT, C + 1], F32, tag="x32t", bufs=2)
                nc.scalar.copy(x32t[:, :C], xtp)
                if bi == 0 and t < 2:
                    nc.vector.memset(x32t[:, C:C + 1], 1.0)
                state[t] = [s16, x32t]

            def stage_a2(t):
                s16, x32t = state[t]
                # top-8 scan (emitted after the NEXT slot's evacs so psum
                # evacuation never queues behind a long scan on DVE); a 2x-mode
                # pairwise max halves the MAX8 input, FIND_INDEX8 matches
                # values on the full array so indices stay in bin space
                m2 = wp.tile([NT, NBINS // 2], F16, tag="m2", bufs=1)
                nc.vector.tensor_tensor(m2, s16[:, :NBINS // 2],
                                        s16[:, NBINS // 2:], ALU.max)
                m4 = wp.tile([NT, NBINS // 4], F16, tag="m4", bufs=1)
                nc.vector.tensor_tensor(m4, m2[:, :NBINS // 4],
                                        m2[:, NBINS // 4:], ALU.max)
                v8 = wp.tile([NT, 8], F16, tag="v8", bufs=2)
                idx8 = wp.tile([NT, 8], U32, tag="idx8", bufs=2)
                nc.vector.max(out=v8, in_=m4)
                nc.vector.max_index(out=idx8, in_max=v8, in_values=s16)
                state[t] = [idx8, x32t]

            def stage_g(t):
                idx8, x32t = state[t]
                gs = []
                for k in range(NCAND):
                    gk = wp.tile([NT, CE], F32, tag=f"g{k}", bufs=2)
                    nc.gpsimd.indirect_dma_start(
                        out=gk, out_offset=None, in_=d["cen_ext"],
                        in_offset=bass.IndirectOffsetOnAxis(
                            ap=idx8[:, k:k + 1], axis=0))
                    gs.append(gk)
                state[t] = [gs, x32t]

            def stage_b(t):
                gs, x32t = state.pop(t)
                # exact re-score: cand0 dot on GpSimd+DVE, cand1 fused on DVE
                p0 = wp.tile([NT, C + 1], F32, tag="p0", bufs=1)
                s0 = wp.tile([NT, 1], F32, tag="s0", bufs=2)
                nc.gpsimd.tensor_mul(p0, gs[0][:, :C + 1], x32t)
                nc.vector.tensor_reduce(s0, p0, AX.X, ALU.add)
                p1 = wp.tile([NT, C + 1], F32, tag="p1", bufs=1)
                s1 = wp.tile([NT, 1], F32, tag="s1", bufs=2)
                nc.vector.scalar_tensor_tensor(
                    out=p1, in0=gs[1][:, :C + 1], scalar=1.0, in1=x32t,
                    op0=ALU.bypass, op1=ALU.mult, accum_out=s1)
                # lin16 = fp16(g0) depends only on the gather; the predicated
                # overwrite with g1 (casting on write) happens on DVE after
                lin16 = wp.tile([SLOT, C], F16, tag="lin16", bufs=2)
                nc.scalar.copy(lin16[:NT], gs[0][:, :C])
                mk = wp.tile([NT, 1], U8, tag="mk", bufs=2)
                nc.vector.tensor_tensor(mk, s1, s0, ALU.is_gt)
                nc.vector.copy_predicated(lin16[:NT], mk.to_broadcast([NT, C]),
                                          gs[1][:, :C])
                # lin channel-major via xbar DMA transpose (Scalar HWDGE ring)
                tsl = slice(t * SLOT, (t + 1) * SLOT)
                nc.scalar.dma_start_transpose(
                    linT[:, :, tsl], lin16)
                # spk residual for this slot: in-place fp16 subtract (2x DVE
                # mode) + fp8 cast on ScalarE; pad columns carry garbage that
                # downstream slices never read
                nc.vector.tensor_tensor(fs16[:, :, tsl], fs16[:, :, tsl],
                                        linT[:, :, tsl], ALU.subtract)
                nc.scalar.copy(spk8[:, :, tsl], fs16[:, :, tsl])

            # software skew: scans one slot behind the coarse matmuls so psum
            # evacs never queue behind a scan; gathers lead their rescue
            stage_a1(0)
            stage_a1(1)
            stage_a2(0)
            stage_g(0)
            stage_a1(2)
            stage_a2(1)
            stage_g(1)
            stage_b(0)
            stage_a1(3)
            stage_a2(2)
            stage_g(2)
            stage_b(1)
            stage_a2(3)
            stage_g(3)
            fstate[bi] += (stage_b,)

        def front_tail(bi):
            batch, blk, feat_b, fs16, linT, spk8, state, stage_b = fstate.pop(bi)
            stage_b(2)
            stage_b(NSLOT - 1)
            saved[bi] = (batch, blk, spk8, linT)

        def mlp(bi):
            batch, blk, spk8, linT = saved.pop(bi)
            out_b = d["out"][batch].rearrange("(g p) t -> p g t", p=128)
            # evac scale for layer L: hs_out / (WS * hs_in); lrelu is
            # positively homogeneous so the scale rides through it.
            h1 = wp.tile([128, 4, BLKW], F8, tag="hA", bufs=1)
            for m in range(4):
                pm = psm.tile([128, BLKW], F32, tag="mlp", bufs=2)
                for g in range(0, KG, 2):
                    nc.tensor.matmul(pm, lhsT=w0[:, g:g + 2, m * 128:(m + 1) * 128],
                                     rhs=spk8[:, g:g + 2, :], perf_mode=PM.DoubleRow,
                                     start=(g == 0), stop=(g == KG - 2))
                nc.scalar.activation(h1[:, m, :], pm, LRELU,
                                     bias=b0[:, m:m + 1], alpha=0.01,
                                     scale=HS[1] / (WS * HS[0]))
            h2 = wp.tile([128, 4, BLKW], F8, tag="hB", bufs=1)
            for m in range(4):
                pm = psm.tile([128, BLKW], F32, tag="mlp", bufs=2)
                for g in range(0, 4, 2):
                    nc.tensor.matmul(pm, lhsT=w1[:, g:g + 2, m * 128:(m + 1) * 128],
                                     rhs=h1[:, g:g + 2, :], perf_mode=PM.DoubleRow,
                                     start=(g == 0), stop=(g == 2))
                nc.scalar.activation(h2[:, m, :], pm, LRELU,
                                     bias=b1[:, m:m + 1], alpha=0.01,
                                     scale=HS[2] / (WS * HS[1]))
            z = wp.tile([128, 2, BLKW], F8, tag="z", bufs=1)
            for m in range(2):
                pm = psm.tile([128, BLKW], F32, tag="mlp", bufs=2)
                for g in range(0, 4, 2):
                    nc.tensor.matmul(pm, lhsT=w2[:, g:g + 2, m * 128:(m + 1) * 128],
                                     rhs=h2[:, g:g + 2, :], perf_mode=PM.DoubleRow,
                                     start=(g == 0), stop=(g == 2))
                nc.scalar.activation(z[:, m, :], pm, AF.Identity,
                                     bias=b2[:, m:m + 1],
                                     scale=HS[3] / (WS * HS[2]))
            d1 = wp.tile([128, 4, BLKW], F8, tag="hA", bufs=1)
            for m in range(4):
                pm = psm.tile([128, BLKW], F32, tag="mlp", bufs=2)
                nc.tensor.matmul(pm, lhsT=v0[:, 0:2, m * 128:(m + 1) * 128],
                                 rhs=z[:, 0:2, :], perf_mode=PM.DoubleRow,
                                 start=True, stop=True)
                nc.scalar.activation(d1[:, m, :], pm, LRELU,
                                     bias=c0[:, m:m + 1], alpha=0.01,
                                     scale=HS[4] / (WS * HS[3]))
            d2 = wp.tile([128, 4, BLKW], F8, tag="hB", bufs=1)
            for m in range(4):
                pm = psm.tile([128, BLKW], F32, tag="mlp", bufs=2)
                for g in range(0, 4, 2):
                    nc.tensor.matmul(pm, lhsT=v1[:, g:g + 2, m * 128:(m + 1) * 128],
                                     rhs=d1[:, g:g + 2, :], perf_mode=PM.DoubleRow,
                                     start=(g == 0), stop=(g == 2))
                nc.scalar.activation(d2[:, m, :], pm, LRELU,
                                     bias=c1[:, m:m + 1], alpha=0.01,
                                     scale=HS[5] / (WS * HS[4]))
            for cc in range(KG):
                pm = psm.tile([128, BLKW], F32, tag="mlp", bufs=2)
                for g in range(0, 4, 2):
                    nc.tensor.matmul(pm, lhsT=v2[:, g:g + 2, cc * 128:(cc + 1) * 128],
                                     rhs=d2[:, g:g + 2, :], perf_mode=PM.DoubleRow,
                                     start=(g == 0), stop=False)
                # += lin*256 (identity matmul folds the codebook row back in)
                nc.tensor.matmul(pm, lhsT=idents, rhs=linT[:, cc, :],
                                 start=False, stop=True)
                occ = wp.tile([128, BLKW], F32, tag="occ", bufs=3)
                nc.scalar.activation(occ, pm, AF.Identity,
                                     bias=c2b[:, cc:cc + 1],
                                     scale=1.0 / (WS * HS[5]))
                nc.sync.dma_start(
                    out=out_b[:, cc, blk * BLKR:(blk + 1) * BLKR]
                        .rearrange("p (t y) -> p t y", y=NT),
                    in_=occ.rearrange("p (t x) -> p t x", x=SLOT)[:, :, :NT])

        nblocks = BPC * NBLK
        for bi in range(nblocks):
            front_head(bi)
            if bi >= 1:
                mlp(bi - 1)
            front_tail(bi)
        mlp(nblocks - 1)


def build_nc():
    nc = bacc.Bacc("TRN2", target_bir_lowering=False, debug=False,
                   enable_asserts=False, num_devices=NCORES)
    d = {}
    d["feat"] = nc.dram_tensor("feat", (BPC, C, T), F32, kind="ExternalInput").ap()
    d["cT16"] = nc.dram_tensor("cT16", (C, NBINS), F16, kind="ExternalInput").ap()
    d["b2rep"] = nc.dram_tensor("b2rep", (128, NBINS), F16,
                                kind="ExternalInput").ap()
    d["cen_ext"] = nc.dram_tensor("cen_ext", (NBINS, CE), F32,
                                  kind="ExternalInput").ap()
    for nm, shp in [("w0T", (C, HID)), ("w1T", (HID, HID)), ("w2T", (HID, EMB)),
                    ("v0T", (EMB, HID)), ("v1T", (HID, HID)), ("v2T", (HID, C))]:
        d[nm] = nc.dram_tensor(nm, shp, F8, kind="ExternalInput").ap()
    for nm, n in [("b0", HID), ("b1", HID), ("b2", EMB),
                  ("c0", HID), ("c1", HID), ("c2b", C)]:
        d[nm] = nc.dram_tensor(nm, (n,), F32, kind="ExternalInput").ap()
    d["out"] = nc.dram_tensor("out", (BPC, C, T), F32, kind="ExternalOutput").ap()

    with tile.TileContext(nc) as tc:
        _build_body(nc, tc, d)
    nc.compile()
    return nc


def _prep_shared(centroid, ew0, eb0, ew1, eb1, ew2, eb2, dw0, db0, dw1, db1,
                 dw2, db2):
    import ml_dtypes
    E4 = ml_dtypes.float8_e4m3fn
    cen = np.asarray(centroid, np.float32)
    c_norm = (cen.astype(np.float64) ** 2).sum(1)
    bias32 = (-0.5 * c_norm).astype(np.float32)
    bias_c = bias32 - np.float32(bias32.mean())
    b2row = (bias_c / 8.0).astype(np.float16)
    cen_ext = np.zeros((NBINS, CE), np.float32)
    cen_ext[:, :C] = cen
    cen_ext[:, C] = bias32
    cen_ext[:, C + 1] = 1.0

    def w8(w):
        return np.ascontiguousarray(
            np.asarray(w, np.float32).T * WS).astype(E4)

    shared = {
        "cT16": np.ascontiguousarray(cen.T).astype(np.float16),
        "b2rep": np.broadcast_to(b2row, (128, NBINS)).copy(),
        "cen_ext": cen_ext,
        "w0T": w8(ew0), "w1T": w8(ew1), "w2T": w8(ew2),
        "v0T": w8(dw0), "v1T": w8(dw1), "v2T": w8(dw2),
        "b0": np.asarray(eb0, np.float32) * np.float32(HS[1]),
        "b1": np.asarray(eb1, np.float32) * np.float32(HS[2]),
        "b2": np.asarray(eb2, np.float32) * np.float32(HS[3]),
        "c0": np.asarray(db0, np.float32) * np.float32(HS[4]),
        "c1": np.asarray(db1, np.float32) * np.float32(HS[5]),
        "c2b": np.asarray(db2, np.float32),
    }
    return shared


def _get_nc():
    if "nc" not in _CACHE:
        _CACHE["nc"] = build_nc()
    return _CACHE["nc"]


def run(inputs, trace=False):
    feature = np.ascontiguousarray(np.asarray(inputs["feature"], np.float32))
    shared = _prep_shared(**{k: v for k, v in inputs.items() if k != "feature"})
    nc = _get_nc()
    in_maps = []
    for c in range(NCORES):
        m = dict(shared)
        m["feat"] = np.ascontiguousarray(feature[c * BPC:(c + 1) * BPC])
        in_maps.append(m)
    kw = {}
    if trace:
        kw = dict(trace=True, trace_cores=list(range(NCORES)))
    res = bass_utils.run_bass_kernel_spmd(nc, in_maps, core_ids=list(range(NCORES)),
                                          **kw)
    out = np.empty((B, C, T), np.float32)
    for c in range(NCORES):
        out[c * BPC:(c + 1) * BPC] = res.results[c]["out"]
    return out, res


def kernel(**inputs) -> np.ndarray:
    out, _ = run(inputs, trace=False)
    return out


# revision 19
# speedup vs baseline: 1.0734x; 1.0734x over previous
"""Trainium2 Bass kernel for nn_M02SameVQ (VQ codebook match + refiner MLP).

v6-final (8 NeuronCores, data-parallel over batch; 849us -> 745us):
 - Coarse scores s = x.c in fp16 on the PE (fp32 PSUM accum), weight-
   stationary over two 4-bank PSUM phases (fewer weight reloads); the
   -0.5||c||^2 bias and the 1/8 score scale fuse into the DVE PSUM
   evacuation (scalar_tensor_tensor: out = psum*0.125 + bias_rep), so the
   PE runs no per-(slot,j) K=2 bias matmuls at all.
 - Top-8 scan (MAX8 + FIND_INDEX8) is emitted one slot behind the coarse
   matmuls so PSUM evacuations never queue behind a 4.4us scan on DVE.
 - Top-2 candidates rescued exactly: indirect row gathers from the fp32
   extended codebook, dots via GpSimd mul + DVE reduce (cand0) and a DVE
   fused mul+row-sum (cand1); winner selected in-place with a predicated
   copy, cast fp16 on ScalarE, moved channel-major by one xbar DMA
   transpose per slot on the Scalar HWDGE ring (feature loads and output
   stores ride the Sync ring, so a rescue-blocked transpose cannot
   head-of-line block them).
 - spk residual per slot: in-place fp16 subtract (2x DVE mode) + fp8 cast
   on ScalarE.
 - Refiner MLP in fp8(e4m3) DoubleRow perf mode (2x PE throughput);
   per-layer power-of-two activation scales and the x16 weight scale fold
   into the activation evacuations (leaky-relu is positively homogeneous);
   lin folds back in the output layer via a x256-scaled fp16 identity
   matmul and db2 is applied as a per-partition activation bias.
 - Accuracy: top-2-of-fp16 + exact fp32 re-score gives 0-1 wrong picks on
   this data (rel err 3.4e-4 measured, tolerance 2e-2); fp8 MLP error is
   negligible because ||spk_dec|| is only 0.3% of the output norm.
"""

import numpy as np
import concourse.tile as tile
import concourse.mybir as mybir
from concourse import bacc, bass, bass_utils
from concourse.masks import make_identity

F32 = mybir.dt.float32
F16 = mybir.dt.float16
F8 = mybir.dt.float8e4
U8 = mybir.dt.uint8
U32 = mybir.dt.uint32
AF = mybir.ActivationFunctionType
ALU = mybir.AluOpType
AX = mybir.AxisListType
PM = mybir.MatmulPerfMode
LRELU = AF.Lrelu

B, C, T = 16, 1024, 1500
NBINS, HID, EMB = 4096, 512, 256
NCORES = 8
BPC = B // NCORES          # batches per core
NT = 125                   # real tokens per slot
SLOT = 128                 # token slot width (3 pad columns)
NSLOT = 4                  # slots per block
BLKR = NT * NSLOT          # real tokens per block (500)
BLKW = SLOT * NSLOT        # slot columns per block (512)
NBLK = T // BLKR           # blocks per batch (3)
CE = 1088                  # ext codebook row: 1024 cen | -0.5||c||^2 | 1.0 | pad
KG = C // 128              # 8 contraction chunks
NJ = NBINS // 512          # 8 bin-tiles
NCAND = 2                  # rescued candidates (top-2; top-3 exact on this data)
WS = 16.0                  # fp8 weight scale
# per-layer fp8 input activation scales: spk, h1, h2, z, d1, d2
HS = [1.0, 4.0, 8.0, 16.0, 8.0, 16.0]

_CACHE = {}


def _build_body(nc, tc, d):
    with tc.tile_pool(name="const", bufs=1) as cp, \
         tc.tile_pool(name="work", bufs=1) as wp, \
         tc.tile_pool(name="psd", bufs=1, space="PSUM") as psd, \
         tc.tile_pool(name="psx", bufs=1, space="PSUM") as psx, \
         tc.tile_pool(name="psm", bufs=1, space="PSUM") as psm:

        # ---- resident constants (coarse-path first so PE can start early) ----
        b2rep = cp.tile([128, NBINS], F16)
        nc.scalar.dma_start(out=b2rep, in_=d["b2rep"])
        cs16 = []
        for q in range(4):
            cq = cp.tile([128, KG, 1024], F16, tag=f"cs{q}")
            eng = nc.scalar if q % 2 == 0 else nc.sync
            eng.dma_start(
                out=cq,
                in_=d["cT16"][:, q * 1024:(q + 1) * 1024]
                    .rearrange("(g p) b -> p g b", p=128))
            cs16.append(cq)
        ident = cp.tile([128, 128], F32)
        make_identity(nc, ident)
        ident16 = cp.tile([128, 128], F16)
        nc.vector.tensor_copy(out=ident16, in_=ident)
        idents = cp.tile([128, 128], F16)
        nc.scalar.activation(idents, ident, AF.Copy, scale=256.0)

        w0 = cp.tile([128, KG, HID], F8)
        nc.scalar.dma_start(out=w0, in_=d["w0T"].rearrange("(g p) h -> p g h", p=128))
        w1 = cp.tile([128, 4, HID], F8)
        nc.scalar.dma_start(out=w1, in_=d["w1T"].rearrange("(g p) h -> p g h", p=128))
        w2 = cp.tile([128, 4, EMB], F8)
        nc.scalar.dma_start(out=w2, in_=d["w2T"].rearrange("(g p) h -> p g h", p=128))
        v0 = cp.tile([128, 2, HID], F8)
        nc.scalar.dma_start(out=v0, in_=d["v0T"].rearrange("(g p) h -> p g h", p=128))
        v1 = cp.tile([128, 4, HID], F8)
        nc.scalar.dma_start(out=v1, in_=d["v1T"].rearrange("(g p) h -> p g h", p=128))
        v2 = cp.tile([128, 4, C], F8)
        nc.scalar.dma_start(out=v2, in_=d["v2T"].rearrange("(g p) h -> p g h", p=128))
        b0 = cp.tile([128, 4], F32)
        nc.scalar.dma_start(out=b0, in_=d["b0"].rearrange("(g p) -> p g", p=128))
        b1 = cp.tile([128, 4], F32)
        nc.scalar.dma_start(out=b1, in_=d["b1"].rearrange("(g p) -> p g", p=128))
        b2 = cp.tile([128, 2], F32)
        nc.scalar.dma_start(out=b2, in_=d["b2"].rearrange("(g p) -> p g", p=128))
        c0 = cp.tile([128, 4], F32)
        nc.scalar.dma_start(out=c0, in_=d["c0"].rearrange("(g p) -> p g", p=128))
        c1 = cp.tile([128, 4], F32)
        nc.scalar.dma_start(out=c1, in_=d["c1"].rearrange("(g p) -> p g", p=128))
        c2b = cp.tile([128, KG], F32)
        nc.scalar.dma_start(out=c2b, in_=d["c2b"].rearrange("(g p) -> p g", p=128))

        saved = {}
        fstate = {}

        def front_head(bi):
            batch, blk = divmod(bi, NBLK)
            feat_b = d["feat"][batch].rearrange("(g p) t -> p g t", p=128)
            fs16 = wp.tile([128, KG, BLKW], F16, tag="fs16", bufs=2)
            linT = wp.tile([128, KG, BLKW], F16, tag="linT", bufs=2)
            spk8 = wp.tile([128, KG, BLKW], F8, tag="spk8", bufs=2)
            state = {}
            fstate[bi] = (batch, blk, feat_b, fs16, linT, spk8, state)

            def stage_a1(t):
                tok0 = blk * BLKR + t * NT
                csl = slice(t * SLOT, t * SLOT + NT)
                fs32t = wp.tile([128, KG, NT], F32, tag="fs32t", bufs=2)
                nc.sync.dma_start(out=fs32t, in_=feat_b[:, :, tok0:tok0 + NT])
                nc.scalar.copy(fs16[:, :, csl], fs32t)

                # coarse fp16 scores, weight-stationary over 4-bank phases;
                # evac fuses 1/8 scale + bin bias on DVE
                s16 = wp.tile([NT, NBINS], F16, tag="s16", bufs=2)
                for ph in range(2):
                    pjs = []
                    for _i in range(4):
                        pj = psd.tile([NT, 512], F32, tag="dist", bufs=4)
                        pjs.append(pj)
                    for g in range(KG):
                        for i, pj in enumerate(pjs):
                            j = ph * 4 + i
                            nc.tensor.matmul(pj, lhsT=fs16[:, g, csl],
                                             rhs=cs16[j // 2][:, g,
                                                              (j % 2) * 512:
                                                              (j % 2) * 512 + 512],
                                             start=(g == 0), stop=(g == KG - 1))
                    for i, pj in enumerate(pjs):
                        j = ph * 4 + i
                        nc.vector.scalar_tensor_tensor(
                            out=s16[:, j * 512:(j + 1) * 512], in0=pj,
                            scalar=0.125, in1=b2rep[:NT, j * 512:(j + 1) * 512],
                            op0=ALU.mult, op1=ALU.add)

                # x_t via PE transpose, evacuated to SBUF
                xtp = psx.tile([NT, C], F32, tag="xt", bufs=1)
                for g in range(KG):
                    nc.tensor.transpose(xtp[:, g * 128:(g + 1) * 128],
                                        fs32t[:, g, :], ident)
                x32t = wp.tile([NT, C + 1], F32, tag="x32t", bufs=2)
                nc.scalar.copy(x32t[:, :C], xtp)
                if bi == 0 and t < 2:
                    nc.vector.memset(x32t[:, C:C + 1], 1.0)
                state[t] = [s16, x32t]

            def stage_a2(t):
                s16, x32t = state[t]
                # top-8 scan (emitted after the NEXT slot's evacs so psum
                # evacuation never queues behind a long scan on DVE); a 2x-mode
                # pairwise max halves the MAX8 input, FIND_INDEX8 matches
                # values on the full array so indices stay in bin space
                m2 = wp.tile([NT, NBINS // 2], F16, tag="m2", bufs=1)
                nc.vector.tensor_tensor(m2, s16[:, :NBINS // 2],
                                        s16[:, NBINS // 2:], ALU.max)
                v8 = wp.tile([NT, 8], F16, tag="v8", bufs=2)
                idx8 = wp.tile([NT, 8], U32, tag="idx8", bufs=2)
                nc.vector.max(out=v8, in_=m2)
                nc.vector.max_index(out=idx8, in_max=v8, in_values=s16)
                state[t] = [idx8, x32t]

            def stage_g(t):
                idx8, x32t = state[t]
                gs = []
                for k in range(NCAND):
                    gk = wp.tile([NT, CE], F32, tag=f"g{k}", bufs=2)
                    nc.gpsimd.indirect_dma_start(
                        out=gk, out_offset=None, in_=d["cen_ext"],
                        in_offset=bass.IndirectOffsetOnAxis(
                            ap=idx8[:, k:k + 1], axis=0))
                    gs.append(gk)
                state[t] = [gs, x32t]

            def stage_b(t):
                gs, x32t = state.pop(t)
                # exact re-score: cand0 dot on GpSimd+DVE, cand1 fused on DVE
                p0 = wp.tile([NT, C + 1], F32, tag="p0", bufs=1)
                s0 = wp.tile([NT, 1], F32, tag="s0", bufs=2)
                nc.gpsimd.tensor_mul(p0, gs[0][:, :C + 1], x32t)
                nc.vector.tensor_reduce(s0, p0, AX.X, ALU.add)
                p1 = wp.tile([NT, C + 1], F32, tag="p1", bufs=1)
                s1 = wp.tile([NT, 1], F32, tag="s1", bufs=2)
                nc.vector.scalar_tensor_tensor(
                    out=p1, in0=gs[1][:, :C + 1], scalar=1.0, in1=x32t,
                    op0=ALU.bypass, op1=ALU.mult, accum_out=s1)
                # lin16 = fp16(g0) depends only on the gather; the predicated
                # overwrite with g1 (casting on write) happens on DVE after
                lin16 = wp.tile([SLOT, C], F16, tag="lin16", bufs=2)
                nc.scalar.copy(lin16[:NT], gs[0][:, :C])
                mk = wp.tile([NT, 1], U8, tag="mk", bufs=2)
                nc.vector.tensor_tensor(mk, s1, s0, ALU.is_gt)
                nc.vector.copy_predicated(lin16[:NT], mk.to_broadcast([NT, C]),
                                          gs[1][:, :C])
                # lin channel-major via xbar DMA transpose (Scalar HWDGE ring)
                tsl = slice(t * SLOT, (t + 1) * SLOT)
                nc.scalar.dma_start_transpose(
                    linT[:, :, tsl], lin16)
                # spk residual for this slot: in-place fp16 subtract (2x DVE
                # mode) + fp8 cast on ScalarE; pad columns carry garbage that
                # downstream slices never read
                nc.vector.tensor_tensor(fs16[:, :, tsl], fs16[:, :, tsl],
                                        linT[:, :, tsl], ALU.subtract)
                nc.scalar.copy(spk8[:, :, tsl], fs16[:, :, tsl])

            # software skew: scans one slot behind the coarse matmuls so psum
            # evacs never queue behind a scan; gathers lead their rescue
            stage_a1(0)
            stage_a1(1)
            stage_a2(0)
            stage_g(0)
            stage_a1(2)
            stage_a2(1)
            stage_g(1)
            stage_b(0)
            stage_a1(3)
            stage_a2(2)
            stage_g(2)
            stage_b(1)
            stage_a2(3)
            stage_g(3)
            fstate[bi] += (stage_b,)

        def front_tail(bi):
            batch, blk, feat_b, fs16, linT, spk8, state, stage_b = fstate.pop(bi)
            stage_b(2)
            stage_b(NSLOT - 1)
            saved[bi] = (batch, blk, spk8, linT)

        def mlp(bi):
            batch, blk, spk8, linT = saved.pop(bi)
            out_b = d["out"][batch].rearrange("(g p) t -> p g t", p=128)
            # evac scale for layer L: hs_out / (WS * hs_in); lrelu is
            # positively homogeneous so the scale rides through it.
            h1 = wp.tile([128, 4, BLKW], F8, tag="hA", bufs=1)
            for m in range(4):
                pm = psm.tile([128, BLKW], F32, tag="mlp", bufs=2)
                for g in range(0, KG, 2):
                    nc.tensor.matmul(pm, lhsT=w0[:, g:g + 2, m * 128:(m + 1) * 128],
                                     rhs=spk8[:, g:g + 2, :], perf_mode=PM.DoubleRow,
                                     start=(g == 0), stop=(g == KG - 2))
                nc.scalar.activation(h1[:, m, :], pm, LRELU,
                                     bias=b0[:, m:m + 1], alpha=0.01,
                                     scale=HS[1] / (WS * HS[0]))
            h2 = wp.tile([128, 4, BLKW], F8, tag="hB", bufs=1)
            for m in range(4):
                pm = psm.tile([128, BLKW], F32, tag="mlp", bufs=2)
                for g in range(0, 4, 2):
                    nc.tensor.matmul(pm, lhsT=w1[:, g:g + 2, m * 128:(m + 1) * 128],
                                     rhs=h1[:, g:g + 2, :], perf_mode=PM.DoubleRow,
                                     start=(g == 0), stop=(g == 2))
                nc.scalar.activation(h2[:, m, :], pm, LRELU,
                                     bias=b1[:, m:m + 1], alpha=0.01,
                                     scale=HS[2] / (WS * HS[1]))
            z = wp.tile([128, 2, BLKW], F8, tag="z", bufs=1)
            for m in range(2):
                pm = psm.tile([128, BLKW], F32, tag="mlp", bufs=2)
                for g in range(0, 4, 2):
                    nc.tensor.matmul(pm, lhsT=w2[:, g:g + 2, m * 128:(m + 1) * 128],
                                     rhs=h2[:, g:g + 2, :], perf_mode=PM.DoubleRow,
                                     start=(g == 0), stop=(g == 2))
                nc.scalar.activation(z[:, m, :], pm, AF.Identity,
                                     bias=b2[:, m:m + 1],
                                     scale=HS[3] / (WS * HS[2]))
            d1 = wp.tile([128, 4, BLKW], F8, tag="hA", bufs=1)
            for m in range(4):
                pm = psm.tile([128, BLKW], F32, tag="mlp", bufs=2)
                nc.tensor.matmul(pm, lhsT=v0[:, 0:2, m * 128:(m + 1) * 128],
                                 rhs=z[:, 0:2, :], perf_mode=PM.DoubleRow,
                                 start=True, stop=True)
                nc.scalar.activation(d1[:, m, :], pm, LRELU,
                                     bias=c0[:, m:m + 1], alpha=0.01,
                                     scale=HS[4] / (WS * HS[3]))
            d2 = wp.tile([128, 4, BLKW], F8, tag="hB", bufs=1)
            for m in range(4):
                pm = psm.tile([128, BLKW], F32, tag="mlp", bufs=2)
                for g in range(0, 4, 2):
                    nc.tensor.matmul(pm, lhsT=v1[:, g:g + 2, m * 128:(m + 1) * 128],
                                     rhs=d1[:, g:g + 2, :], perf_mode=PM.DoubleRow,
                                     start=(g == 0), stop=(g == 2))
                nc.scalar.activation(d2[:, m, :], pm, LRELU,
                                     bias=c1[:, m:m + 1], alpha=0.01,
                                     scale=HS[5] / (WS * HS[4]))
            for cc in range(KG):
                pm = psm.tile([128, BLKW], F32, tag="mlp", bufs=2)
                for g in range(0, 4, 2):
                    nc.tensor.matmul(pm, lhsT=v2[:, g:g + 2, cc * 128:(cc + 1) * 128],
                                     rhs=d2[:, g:g + 2, :], perf_mode=PM.DoubleRow,
                                     start=(g == 0), stop=False)
                # += lin*256 (identity matmul folds the codebook row back in)
                nc.tensor.matmul(pm, lhsT=idents, rhs=linT[:, cc, :],
                                 start=False, stop=True)
                occ = wp.tile([128, BLKW], F32, tag="occ", bufs=3)
                nc.scalar.activation(occ, pm, AF.Identity,
                                     bias=c2b[:, cc:cc + 1],
                                     scale=1.0 / (WS * HS[5]))
                nc.sync.dma_start(
                    out=out_b[:, cc, blk * BLKR:(blk + 1) * BLKR]
                        .rearrange("p (t y) -> p t y", y=NT),
                    in_=occ.rearrange("p (t x) -> p t x", x=SLOT)[:, :, :NT])

        nblocks = BPC * NBLK
        for bi in range(nblocks):
            front_head(bi)
            if bi >= 1:
                mlp(bi - 1)
            front_tail(bi)
        mlp(nblocks - 1)


def build_nc():
    nc = bacc.Bacc("TRN2", target_bir_lowering=False, debug=False,
                   enable_asserts=False, num_devices=NCORES)
    d = {}
    d["feat"] = nc.dram_tensor("feat", (BPC, C, T), F32, kind="ExternalInput").ap()
    d["cT16"] = nc.dram_tensor("cT16", (C, NBINS), F16, kind="ExternalInput").ap()
    d["b2rep"] = nc.dram_tensor("b2rep", (128, NBINS), F16,
                                kind="ExternalInput").ap()
    d["cen_ext"] = nc.dram_tensor("cen_ext", (NBINS, CE), F32,
                                  kind="ExternalInput").ap()
    for nm, shp in [("w0T", (C, HID)), ("w1T", (HID, HID)), ("w2T", (HID, EMB)),
                    ("v0T", (EMB, HID)), ("v1T", (HID, HID)), ("v2T", (HID, C))]:
        d[nm] = nc.dram_tensor(nm, shp, F8, kind="ExternalInput").ap()
    for nm, n in [("b0", HID), ("b1", HID), ("b2", EMB),
                  ("c0", HID), ("c1", HID), ("c2b", C)]:
        d[nm] = nc.dram_tensor(nm, (n,), F32, kind="ExternalInput").ap()
    d["out"] = nc.dram_tensor("out", (BPC, C, T), F32, kind="ExternalOutput").ap()

    with tile.TileContext(nc) as tc:
        _build_body(nc, tc, d)
    nc.compile()
    return nc


def _prep_shared(centroid, ew0, eb0, ew1, eb1, ew2, eb2, dw0, db0, dw1, db1,
                 dw2, db2):
    import ml_dtypes
    E4 = ml_dtypes.float8_e4m3fn
    cen = np.asarray(centroid, np.float32)
    c_norm = (cen.astype(np.float64) ** 2).sum(1)
    bias32 = (-0.5 * c_norm).astype(np.float32)
    bias_c = bias32 - np.float32(bias32.mean())
    b2row = (bias_c / 8.0).astype(np.float16)
    cen_ext = np.zeros((NBINS, CE), np.float32)
    cen_ext[:, :C] = cen
    cen_ext[:, C] = bias32
    cen_ext[:, C + 1] = 1.0

    def w8(w):
        return np.ascontiguousarray(
            np.asarray(w, np.float32).T * WS).astype(E4)

    shared = {
        "cT16": np.ascontiguousarray(cen.T).astype(np.float16),
        "b2rep": np.broadcast_to(b2row, (128, NBINS)).copy(),
        "cen_ext": cen_ext,
        "w0T": w8(ew0), "w1T": w8(ew1), "w2T": w8(ew2),
        "v0T": w8(dw0), "v1T": w8(dw1), "v2T": w8(dw2),
        "b0": np.asarray(eb0, np.float32) * np.float32(HS[1]),
        "b1": np.asarray(eb1, np.float32) * np.float32(HS[2]),
        "b2": np.asarray(eb2, np.float32) * np.float32(HS[3]),
        "c0": np.asarray(db0, np.float32) * np.float32(HS[4]),
        "c1": np.asarray(db1, np.float32) * np.float32(HS[5]),
        "c2b": np.asarray(db2, np.float32),
    }
    return shared


def _get_nc():
    if "nc" not in _CACHE:
        _CACHE["nc"] = build_nc()
    return _CACHE["nc"]


def run(inputs, trace=False):
    feature = np.ascontiguousarray(np.asarray(inputs["feature"], np.float32))
    shared = _prep_shared(**{k: v for k, v in inputs.items() if k != "feature"})
    nc = _get_nc()
    in_maps = []
    for c in range(NCORES):
        m = dict(shared)
        m["feat"] = np.ascontiguousarray(feature[c * BPC:(c + 1) * BPC])
        in_maps.append(m)
    kw = {}
    if trace:
        kw = dict(trace=True, trace_cores=list(range(NCORES)))
    res = bass_utils.run_bass_kernel_spmd(nc, in_maps, core_ids=list(range(NCORES)),
                                          **kw)
    out = np.empty((B, C, T), np.float32)
    for c in range(NCORES):
        out[c * BPC:(c + 1) * BPC] = res.results[c]["out"]
    return out, res


def kernel(**inputs) -> np.ndarray:
    out, _ = run(inputs, trace=False)
    return out
